# revision 1
# baseline (speedup 1.0000x reference)
"""Trainium2 Bass kernel for nn_DSSM_57629871178390 (dual-stream Mamba/DSSM block).

Sharding: d_inner=256 split 8 ways across cores (32 channels each). The
selective scan runs on the DVE via tensor_tensor_scan with 128-partition
tiles laid out as (n=16) x (d8): 4 d-groups x (B*K)=6 pairs per core.
Cross-core reductions (AllReduce): x_dbl partials, LN stats, out_proj
partials.

Self-contained: hardcodes all shapes; imports numpy + concourse (available
at /opt/trn_rl_repo inside the container).
"""

import sys
from dataclasses import dataclass

import numpy as np

if "/opt/trn_rl_repo" not in sys.path:
    sys.path.insert(0, "/opt/trn_rl_repo")

import ml_dtypes  # noqa: E402

import concourse.bass as bass  # noqa: E402
import concourse.bacc as bacc  # noqa: E402
import concourse.tile as tile  # noqa: E402
import concourse.mybir as mybir  # noqa: E402
from concourse import bass_utils  # noqa: E402

F32 = mybir.dt.float32
F32R = mybir.dt.float32r
BF16 = mybir.dt.bfloat16
AF = mybir.ActivationFunctionType
OP = mybir.AluOpType
NPBF16 = ml_dtypes.bfloat16


@dataclass(frozen=True)
class Cfg:
    B: int = 2
    C: int = 128
    H: int = 64
    W: int = 64
    D: int = 256          # d_inner
    N: int = 16           # d_state
    R: int = 8            # dt_rank
    K: int = 3
    n_cores: int = 8
    silu_lut: bool = True   # False: sigmoid+mul (CoreSim lacks Silu)
    fake_cc: bool = False   # replace collectives with plain DMA (profiling)
    no_rep: bool = False    # profiling: static tiles instead of replicate DMAs
    no_lut: bool = False    # profiling: Copy instead of Sigmoid/Ln/Exp
    no_scan: bool = False   # profiling: tensor_copy instead of scan
    phases: str = "fxdsl"   # profiling: which phases to build

    @property
    def L(self):
        return self.H * self.W

    @property
    def L2(self):
        return 2 * self.L

    @property
    def Dsh(self):
        return self.D // self.n_cores

    @property
    def DG(self):
        return 128 // self.N       # d-values per 128-partition tile (8)

    @property
    def G(self):
        return self.Dsh // self.DG  # d-groups per core

    @property
    def Tc(self):
        return min(1024, self.L2)   # scan chunk

    @property
    def TOK(self):
        return min(512, self.L)     # in_proj token chunk

    @property
    def CR(self):
        return max(1, min(512 // self.W, self.H))  # conv rows per chunk

    @property
    def TcL(self):
        return min(512, self.L)     # LN/out_proj chunk


CFG = Cfg()
MMF = 512  # max moving free dim per matmul


# ---------------------------------------------------------------------------
# Host-side preparation of per-core input maps
# ---------------------------------------------------------------------------

def host_prep(cfg: Cfg, inputs: dict) -> list:
    B, C, H, W = cfg.B, cfg.C, cfg.H, cfg.W
    D, N, R, K = cfg.D, cfg.N, cfg.R, cfg.K
    Dsh, G, DG, L = cfg.Dsh, cfg.G, cfg.DG, cfg.L

    ms = np.asarray(inputs["ms"], np.float32).reshape(B, C, L)
    pan = np.asarray(inputs["pan"], np.float32).reshape(B, C, L)
    w_ms = np.asarray(inputs["in_proj_ms_w"], np.float32)
    w_pan = np.asarray(inputs["in_proj_pan_w"], np.float32)
    cw_ms = np.asarray(inputs["conv_ms_w"], np.float32)
    cb_ms = np.asarray(inputs["conv_ms_b"], np.float32)
    cw_pan = np.asarray(inputs["conv_pan_w"], np.float32)
    cb_pan = np.asarray(inputs["conv_pan_b"], np.float32)
    xpw = np.asarray(inputs["x_proj_weight"], np.float32)
    dtw = np.asarray(inputs["dt_projs_weight"], np.float32)
    dtb = np.asarray(inputs["dt_projs_bias"], np.float32)
    A_logs = np.asarray(inputs["A_logs"], np.float32)
    Ds = np.asarray(inputs["Ds"], np.float32)
    lnw_v = np.asarray(inputs["out_norm_vis_w"], np.float32)
    lnb_v = np.asarray(inputs["out_norm_vis_b"], np.float32)
    lnw_i = np.asarray(inputs["out_norm_inf_w"], np.float32)
    lnb_i = np.asarray(inputs["out_norm_inf_b"], np.float32)
    opw_ms = np.asarray(inputs["out_proj_ms_w"], np.float32)
    opw_pan = np.asarray(inputs["out_proj_pan_w"], np.float32)

    # dt is carried NEGATED (ndt = -softplus(.) = ln(sigmoid(-.))), so A is
    # stored positive (exp(dt*A) = exp(ndt*(+A)) with A>0... A_eff = -A_true)
    A = np.exp(A_logs).reshape(K, D, N)
    Dsum = Ds.reshape(K, D).sum(0)

    # partition layout: p = dd*N + n  (dd-major); -1 entries undo the
    # negated-dt sign carried through b = (-dt*x)*B
    red = np.zeros((128, G * Dsh), NPBF16)
    for g in range(G):
        for p in range(128):
            red[p, g * Dsh + DG * g + (p // N)] = -1.0

    selst_h = np.zeros((128, 2, 4 * B), NPBF16)
    for pi in range(128):
        selst_h[pi, 0, pi // Dsh] = 1.0          # S1 rows
        selst_h[pi, 1, 2 * B + pi // Dsh] = 1.0  # S2 rows
    seldd = np.zeros((DG, 128), NPBF16)
    for pi in range(128):
        seldd[pi // N, pi] = 1.0

    shared = {
        "seldd": seldd,
        "selst": selst_h.reshape(128, 2 * 4 * B),
        "msf": ms.astype(NPBF16),
        "panf": pan.astype(NPBF16),
        "red": red,
    }

    maps = []
    for c in range(cfg.n_cores):
        dsl = slice(Dsh * c, Dsh * (c + 1))
        m = dict(shared)
        # cols 0:Dsh = z rows (W[D+dsl]), cols Dsh:2Dsh = x rows (W[dsl])
        m["w_in_ms"] = np.ascontiguousarray(
            np.concatenate([w_ms[D + Dsh * c: D + Dsh * (c + 1)].T,
                            w_ms[dsl].T], 1)).astype(NPBF16)
        m["w_in_pan"] = np.ascontiguousarray(
            np.concatenate([w_pan[D + Dsh * c: D + Dsh * (c + 1)].T,
                            w_pan[dsl].T], 1)).astype(NPBF16)
        cd_ms = np.zeros((Dsh, 9 * Dsh), NPBF16)
        cd_pan = np.zeros((Dsh, 9 * Dsh), NPBF16)
        for t in range(9):
            ky, kx = t // 3, t % 3
            for i in range(Dsh):
                cd_ms[i, t * Dsh + i] = cw_ms[Dsh * c + i, 0, ky, kx]
                cd_pan[i, t * Dsh + i] = cw_pan[Dsh * c + i, 0, ky, kx]
        m["conv_d_ms"] = cd_ms
        m["conv_d_pan"] = cd_pan
        m["conv_b_ms"] = cb_ms[dsl].reshape(Dsh, 1).astype(np.float32)
        m["conv_b_pan"] = cb_pan[dsl].reshape(Dsh, 1).astype(np.float32)
        m["xproj_T"] = np.ascontiguousarray(
            xpw[:, :, dsl].transpose(2, 0, 1).reshape(Dsh, K * (R + 2 * N))
        ).astype(NPBF16)
        m["dtw_T"] = np.ascontiguousarray(
            dtw[:, dsl, :].transpose(2, 0, 1).reshape(R, K * Dsh)
        ).astype(NPBF16)
        m["dtb"] = np.ascontiguousarray(-dtb[:, dsl].T).astype(np.float32)  # (Dsh, K), negated
        acol = np.zeros((128, K * G), np.float32)
        for k in range(K):
            for g in range(G):
                for p in range(128):
                    dd, n = p // N, p % N
                    acol[p, k * G + g] = A[k, Dsh * c + DG * g + dd, n]
        m["A_col"] = acol
        m["Dsum"] = Dsum[dsl].reshape(Dsh, 1).astype(np.float32)
        m["lnw"] = np.stack([lnw_v[dsl], lnw_i[dsl]], 1).astype(np.float32)  # (Dsh,2)
        m["lnb"] = np.stack([lnb_v[dsl], lnb_i[dsl]], 1).astype(np.float32)
        lw4 = np.zeros((128, 2), np.float32)
        for bs in range(2 * B):
            sle = bs % 2
            lw4[bs * Dsh:(bs + 1) * Dsh, 0] = (lnw_v if sle == 0 else lnw_i)[dsl]
            lw4[bs * Dsh:(bs + 1) * Dsh, 1] = (lnb_v if sle == 0 else lnb_i)[dsl]
        m["lnwb4"] = lw4
        o4 = np.zeros((128, C), NPBF16)
        for bs in range(2 * B):
            sle = bs % 2
            o4[bs * Dsh:(bs + 1) * Dsh] = (
                (opw_ms if sle == 0 else opw_pan)[:, dsl].T).astype(NPBF16)
        m["opw4"] = o4
        m["opw"] = np.ascontiguousarray(
            np.concatenate([opw_ms[:, dsl].T, opw_pan[:, dsl].T], 1)
        ).astype(NPBF16)  # (Dsh, 2C)
        maps.append(m)
    return maps


# ---------------------------------------------------------------------------
# AP helpers
# ---------------------------------------------------------------------------

def rep_sbuf(ap2d, n_inner):
    """SBUF [P, F] slice -> src AP for dest [P*n_inner, F] with each source
    partition repeated n_inner times (dest p = i*n_inner + j reads src i).
    Legal for SBUF: first (partition) pair keeps its nonzero step."""
    pairs = [list(x) for x in ap2d.ap]
    assert len(pairs) == 2
    return bass.AP(tensor=ap2d.tensor, offset=ap2d.offset,
                   ap=[pairs[0], [0, n_inner], pairs[1]])


def rep_dram(tensor_ap, offset, row_stride, n_rows, n_rep, fstep, fcount):
    """DRAM source AP replicating a [n_rows, fcount] block so that dest
    partition p = r_outer*n_rows + r reads row r: iteration
    (rep, row, f) -> [[0, n_rep], [row_stride, n_rows], [fstep, fcount]]."""
    return bass.AP(tensor=tensor_ap.tensor, offset=tensor_ap.offset + offset,
                   ap=[[0, n_rep], [row_stride, n_rows], [fstep, fcount]])


def mm(nc, out_ps, lhsT, rhs, start, stop, maxf=MMF):
    """Matmul with moving-free-dim splitting. out/rhs 2D [P, F]."""
    F = rhs.shape[-1]
    if F <= maxf:
        nc.tensor.matmul(out_ps, lhsT, rhs, start=start, stop=stop,
                         skip_group_check=True)
        return
    assert F % maxf == 0
    for i in range(F // maxf):
        nc.tensor.matmul(out_ps[:, i * maxf:(i + 1) * maxf], lhsT,
                         rhs[:, i * maxf:(i + 1) * maxf], start=start, stop=stop,
                         skip_group_check=True)


# ---------------------------------------------------------------------------
# Bass program builder
# ---------------------------------------------------------------------------


# ---------------------------------------------------------------------------

def build_nc(cfg: Cfg):
    B, C, H, W = cfg.B, cfg.C, cfg.H, cfg.W
    D, N, R, K = cfg.D, cfg.N, cfg.R, cfg.K
    Dsh, G, DG = cfg.Dsh, cfg.G, cfg.DG
    L, L2, Tc = cfg.L, cfg.L2, cfg.Tc
    TOK, CRW, TcL = cfg.TOK, cfg.CR, cfg.TcL
    NCH = L2 // Tc
    PC = min(1024, L2)           # x_dbl phase chunk
    XD = R + 2 * N               # 40
    Hp, Wp = H + 2, W + 2
    groups = [list(range(cfg.n_cores))]
    nLch = L // TcL
    SC = 2 * B * L // 64         # stats cols for [64, SC] view

    nc = bacc.Bacc("TRN2", target_bir_lowering=False, debug=False,
                   enable_asserts=False, num_devices=cfg.n_cores)

    t = {}

    def inp(name, shape, dt):
        t[name] = nc.dram_tensor(name, shape, dt, kind="ExternalInput").ap()

    inp("msf", [B, C, L], BF16)
    inp("panf", [B, C, L], BF16)
    inp("w_in_ms", [C, 2 * Dsh], BF16)    # cols 0:Dsh z-rows, Dsh:2Dsh x-rows
    inp("w_in_pan", [C, 2 * Dsh], BF16)
    inp("conv_d_ms", [Dsh, 9 * Dsh], BF16)
    inp("conv_d_pan", [Dsh, 9 * Dsh], BF16)
    inp("conv_b_ms", [Dsh, 1], F32)
    inp("conv_b_pan", [Dsh, 1], F32)
    inp("xproj_T", [Dsh, K * XD], BF16)
    inp("dtw_T", [R, K * Dsh], BF16)
    inp("dtb", [Dsh, K], F32)             # negated bias
    inp("A_col", [128, K * G], F32)       # positive exp(A_logs)
    inp("Dsum", [Dsh, 1], F32)
    inp("lnw", [Dsh, 2], F32)
    inp("lnb", [Dsh, 2], F32)
    inp("opw", [Dsh, 2 * C], BF16)
    inp("red", [128, G * Dsh], BF16)      # -1 entries
    inp("selst", [128, 2 * 4 * B], BF16)
    inp("seldd", [DG, 128], BF16)
    inp("lnwb4", [128, 2], F32)
    inp("opw4", [128, C], BF16)

    out_ms = nc.dram_tensor("out_ms", [B, C, H, W], F32, kind="ExternalOutput").ap()
    out_pan = nc.dram_tensor("out_pan", [B, C, H, W], F32, kind="ExternalOutput").ap()

    # internal DRAM (Shared collective outputs only supported for >4 cores)
    shsp = "Shared" if cfg.n_cores > 4 else "Local"
    inter_dram = nc.dram_tensor("inter_dram", [Dsh, B, 2, L], BF16, kind="Internal").ap()
    zs_dram = nc.dram_tensor("zs_dram", [Dsh, 2 * B, L], BF16, kind="Internal").ap()
    xdbl_part = nc.dram_tensor("xdbl_part", [B, K * XD, L2], BF16, kind="Internal").ap()
    xdbl_full = nc.dram_tensor("xdbl_full", [B, K * XD, L2], BF16,
                               kind="Internal", addr_space=shsp).ap()
    stats_part = nc.dram_tensor("stats_part", [2, 2 * B, L], F32, kind="Internal").ap()
    stats_full = nc.dram_tensor("stats_full", [2, 2 * B, L], F32,
                                kind="Internal", addr_space=shsp).ap()
    ab_dram = nc.dram_tensor("ab_dram", [2, 2 * B, L], F32, kind="Internal").ap()
    brc_dram = nc.dram_tensor("brc_dram", [2 * N, 2, Tc], BF16, kind="Internal").ap()
    yfin_dram = nc.dram_tensor("yfin_dram", [Dsh, B, L2], BF16, kind="Internal").ap()
    y1_dram = nc.dram_tensor("y1_dram", [Dsh, B, L2], BF16, kind="Internal").ap()
    outp_part = nc.dram_tensor("outp_part", [2, B, C, L], F32, kind="Internal").ap()
    outp_full = nc.dram_tensor("outp_full", [2, B, C, L], F32,
                               kind="Internal").ap()

    # persistent SBUF
    def sb(name, shape, dt):
        return nc.alloc_sbuf_tensor(name, shape, dt).ap()

    dt3_1 = sb("dt3_1", [Dsh, L2], BF16)               # k=1 ndt, full length
    dt_st = sb("dt_st", [Dsh, 2, 2, Tc], BF16)         # per-chunk ndt (k0,k2)
    dtx_st = sb("dtx_st", [Dsh, 2, 2, Tc], BF16)       # per-chunk ndt*x
    # k1 staging slots (free dim): 0 dtRc, 1 dtxRc, 2 ib-mirror-rev
    k1st = sb("k1st", [Dsh, 2, 3, Tc], BF16)
    carry = sb("carry_s", [128, K * G], F32)
    dummy = sb("dummy_rep", [128, Tc], BF16) if cfg.no_rep else None

    w_in_ms = sb("w_in_ms_s", [C, 2 * Dsh], BF16)
    w_in_pan = sb("w_in_pan_s", [C, 2 * Dsh], BF16)
    convd = sb("convd_s", [2 * Dsh, 2, 9, Dsh], BF16)  # diag at rows Dsh:2Dsh
    convb = sb("convb_s", [Dsh, 2], F32)
    xproj_T = sb("xproj_T_s", [Dsh, K, XD], BF16)
    dtw_T = sb("dtw_T_s", [R, K, Dsh], BF16)
    dtb_s = sb("dtb_s", [Dsh, K], F32)
    A_col = sb("A_col_s", [128, K * G], F32)
    Dsum_s = sb("Dsum_s", [Dsh, 1], F32)
    lnw_s = sb("lnw_s", [Dsh, 2], F32)
    lnb_s = sb("lnb_s", [Dsh, 2], F32)
    opw_s = sb("opw_s", [Dsh, 2, C], BF16)
    red_s = sb("red_s", [128, G, Dsh], BF16)
    selst = sb("selst_s", [128, 2, 4 * B], BF16)
    seldd_s = sb("seldd_s", [DG, 128], BF16)
    lnwb4 = sb("lnwb4_s", [128, 2], F32)
    opw4 = sb("opw4_s", [128, C], BF16)

    def load_inter(pool, b, c0, clen, tag):
        """Build interleaved inter chunk [Dsh, clen] covering scan positions
        [c0, c0+clen) from the two contiguous planes, via DVE stride-2 writes."""
        t0 = pool.tile([Dsh, clen // 2], BF16, tag=tag + "p0")
        nc.sync.dma_start(out=t0,
                          in_=inter_dram[:, b, 0, c0 // 2:(c0 + clen) // 2])
        t1 = pool.tile([Dsh, clen // 2], BF16, tag=tag + "p1")
        nc.sync.dma_start(out=t1,
                          in_=inter_dram[:, b, 1, c0 // 2:(c0 + clen) // 2])
        it = pool.tile([Dsh, clen], BF16, tag=tag)
        nc.vector.tensor_copy(out=it[:, 0::2], in_=t0)
        nc.vector.tensor_copy(out=it[:, 1::2], in_=t1)
        return it

    def allreduce(in_ap, out_ap):
        if cfg.fake_cc:
            nc.sync.dma_start(out=out_ap, in_=in_ap)
        else:
            nc.gpsimd.collective_compute(
                "AllReduce", OP.add, replica_groups=groups,
                ins=[in_ap.opt()], outs=[out_ap.opt()])

    with tile.TileContext(nc) as tc:
        # ---- load weights ----
        if dummy is not None:
            nc.vector.memset(dummy, 0.125)
        for dst, srcw in [
            (w_in_ms, t["w_in_ms"]), (w_in_pan, t["w_in_pan"]),
            (convd[Dsh:2 * Dsh, 0], t["conv_d_ms"].rearrange(
                "p (x d) -> p x d", d=Dsh)),
            (convd[Dsh:2 * Dsh, 1], t["conv_d_pan"].rearrange(
                "p (x d) -> p x d", d=Dsh)),
            (convb[:, 0:1], t["conv_b_ms"]), (convb[:, 1:2], t["conv_b_pan"]),
            (xproj_T, t["xproj_T"].rearrange("p (k x) -> p k x", x=XD)),
            (dtw_T, t["dtw_T"].rearrange("p (k d) -> p k d", d=Dsh)),
            (dtb_s, t["dtb"]), (A_col, t["A_col"]), (Dsum_s, t["Dsum"]),
            (lnw_s, t["lnw"]), (lnb_s, t["lnb"]),
            (opw_s, t["opw"].rearrange("p (s c) -> p s c", c=C)),
            (red_s, t["red"].rearrange("p (g d) -> p g d", d=Dsh)),
            (selst, t["selst"].rearrange("p (x m) -> p x m", m=4 * B)),
            (seldd_s, t["seldd"]),
            (lnwb4, t["lnwb4"]), (opw4, t["opw4"]),
        ]:
            nc.sync.dma_start(out=dst, in_=srcw)

        # ================= Phase F: in_proj + conv + silu =================
        if "f" in cfg.phases:
          with tc.tile_pool(name="f_ps", bufs=2, space="PSUM") as f_ps, \
               tc.tile_pool(name="f_cv", bufs=2, space="PSUM") as f_cv, \
               tc.tile_pool(name="f_src", bufs=3) as f_src, \
               tc.tile_pool(name="f_st", bufs=3) as f_st, \
               tc.tile_pool(name="f_xpad", bufs=2) as f_xpad:
              for b in range(B):
                  for s in range(2):
                      srcT = t["msf"] if s == 0 else t["panf"]
                      w_in = w_in_ms if s == 0 else w_in_pan
                      xpad = f_xpad.tile([2 * Dsh, Hp, Wp], BF16, tag="xpad")
                      nc.vector.memset(xpad[Dsh:2 * Dsh], 0.0)
                      for j in range(L // TOK):
                          mt = f_src.tile([C, TOK], BF16, tag="msrc")
                          nc.sync.dma_start(out=mt,
                                            in_=srcT[b, :, j * TOK:(j + 1) * TOK])
                          ps = f_ps.tile([2 * Dsh, TOK], F32, tag="fps")
                          mm(nc, ps, w_in, mt, start=True, stop=True)
                          # x part (rows Dsh:2Dsh) -> xpad interior
                          rpc = TOK // W
                          nc.scalar.copy(
                              out=xpad[Dsh:2 * Dsh,
                                       1 + j * rpc:1 + (j + 1) * rpc, 1:1 + W],
                              in_=ps[Dsh:2 * Dsh, :].rearrange(
                                  "p (r w) -> p r w", w=W))
                          # z part (rows 0:Dsh) -> silu -> staging -> zs_dram
                          zt = f_st.tile([Dsh, TOK], BF16, tag="zst")
                          if cfg.silu_lut:
                              nc.scalar.activation(out=zt, in_=ps[0:Dsh, :],
                                                   func=AF.Silu)
                          else:
                              sgz = f_st.tile([Dsh, TOK], BF16, tag="sgz")
                              nc.scalar.activation(out=sgz, in_=ps[0:Dsh, :],
                                                   func=AF.Sigmoid)
                              nc.vector.tensor_mul(zt, ps[0:Dsh, :], sgz)
                          nc.sync.dma_start(
                              out=zs_dram[:, 2 * b + s, j * TOK:(j + 1) * TOK],
                              in_=zt)
                      # conv: 9 accumulated diag matmuls per row-chunk
                      for j in range(H // CRW):
                          cps = f_cv.tile([Dsh, CRW * W], F32, tag="cps")
                          for tap in range(9):
                              ky, kx = tap // 3, tap % 3
                              rhs = xpad[Dsh:2 * Dsh,
                                         ky + j * CRW: ky + (j + 1) * CRW,
                                         kx:kx + W]
                              nc.tensor.matmul(cps, convd[Dsh:2 * Dsh, s, tap, :],
                                               rhs,
                                               start=(tap == 0), stop=(tap == 8),
                                               skip_group_check=True)
                          # silu(conv + bias) -> staging -> inter_dram (stride 2)
                          ct = f_st.tile([Dsh, CRW * W], BF16, tag="cst")
                          if cfg.silu_lut:
                              nc.scalar.activation(out=ct, in_=cps, func=AF.Silu,
                                                   bias=convb[:, s:s + 1])
                          else:
                              sgc = f_st.tile([Dsh, CRW * W], BF16, tag="sgc")
                              nc.scalar.activation(out=sgc, in_=cps,
                                                   func=AF.Sigmoid,
                                                   bias=convb[:, s:s + 1])
                              nc.vector.scalar_tensor_tensor(
                                  out=ct, in0=cps, scalar=convb[:, s:s + 1],
                                  in1=sgc, op0=OP.add, op1=OP.mult)
                          nc.sync.dma_start(
                              out=inter_dram[:, b, s,
                                             j * CRW * W:(j + 1) * CRW * W],
                              in_=ct)

        # ============ Phase X + k1-dt + scan, per batch ============
        if "x" in cfg.phases:
            with tc.tile_pool(name="x_ps", bufs=2, space="PSUM") as x_ps, \
                 tc.tile_pool(name="x_ib", bufs=3) as x_ib, \
                 tc.tile_pool(name="x_st", bufs=3) as x_st:
                for b in range(B):
                    for j in range(L2 // PC):
                        ibt = load_inter(x_ib, b, j * PC, PC, "xib")
                        for k in range(K):
                            ps = x_ps.tile([XD, PC], F32, tag="xps")
                            mm(nc, ps, xproj_T[:, k, :], ibt,
                               start=True, stop=True)
                            st = x_st.tile([XD, PC], BF16, tag="xst")
                            nc.scalar.copy(out=st, in_=ps)
                            nc.sync.dma_start(
                                out=xdbl_part[b, k * XD:(k + 1) * XD,
                                              j * PC:(j + 1) * PC],
                                in_=st)
            allreduce(xdbl_part, xdbl_full)

        for b in range(B):

            # ---------- k=1 dt over full length ----------
            if "d" in cfg.phases:
              with tc.tile_pool(name="d_ps", bufs=2, space="PSUM") as d_ps, \
                   tc.tile_pool(name="d_st", bufs=3) as d_st:
                  for j in range(L2 // PC):
                      stg = d_st.tile([R, PC], BF16, tag="dstg")
                      nc.sync.dma_start(
                          out=stg, in_=xdbl_full[b, XD:XD + R,
                                                 j * PC:(j + 1) * PC])
                      ps = d_ps.tile([Dsh, PC], F32, tag="dps")
                      mm(nc, ps, dtw_T[:, 1, :], stg, start=True, stop=True)
                      sg = d_st.tile([Dsh, PC], F32, tag="dsg")
                      nc.scalar.activation(out=sg, in_=ps, func=AF.Sigmoid,
                                           bias=dtb_s[:, 1:2], scale=-1.0)
                      nc.scalar.activation(
                          out=dt3_1[:, j * PC:(j + 1) * PC], in_=sg, func=AF.Ln)

            # ---------- Phase S: scan ----------
            dt3_1R = dt3_1[:, ::-1]
            if "s" in cfg.phases:
              with tc.tile_pool(name="s_y_ps", bufs=1, space="PSUM") as s_y_ps, \
                   tc.tile_pool(name="s_d_ps", bufs=1, space="PSUM") as s_d_ps, \
                   tc.tile_pool(name="s_dr_ps", bufs=1, space="PSUM") as s_dr_ps, \
                   tc.tile_pool(name="s_a", bufs=3) as s_a, \
                   tc.tile_pool(name="s_b", bufs=3) as s_b, \
                   tc.tile_pool(name="s_h", bufs=3) as s_h, \
                   tc.tile_pool(name="s_hc", bufs=3) as s_hc, \
                   tc.tile_pool(name="s_rep", bufs=3) as s_rep, \
                   tc.tile_pool(name="s_io", bufs=4) as s_io:
                  for ch in range(NCH):
                      cs = slice(ch * Tc, (ch + 1) * Tc)
                      mcs = slice(L2 - (ch + 1) * Tc, L2 - ch * Tc)
                      par = ch % 2
                      y02_ps = s_y_ps.tile([Dsh, Tc], F32, tag="y02")
                      y1_ps = s_y_ps.tile([Dsh, Tc], F32, tag="y1ps")
                      # chunk loads of inter (fwd + mirror)
                      ibc = load_inter(s_io, b, ch * Tc, Tc, "ibc")
                      ibmc = load_inter(s_io, b, L2 - (ch + 1) * Tc, Tc, "ibmc")
                      # ---- per-chunk k1 reversed staging ----
                      dtRc = k1st[:, par, 0, :]
                      dtxRc = k1st[:, par, 1, :]
                      ibRc = k1st[:, par, 2, :]
                      nc.vector.tensor_copy(out=dtRc, in_=dt3_1R[:, cs])
                      nc.vector.tensor_copy(out=ibRc, in_=ibmc[:, ::-1])
                      nc.vector.tensor_mul(dtxRc, dtRc, ibRc)
                      bcF = s_io.tile([2 * N, Tc], BF16, tag="bcF")
                      nc.sync.dma_start(
                          out=bcF[0:N, :],
                          in_=xdbl_full[b, XD + R:XD + R + N, mcs])
                      nc.sync.dma_start(
                          out=bcF[N:2 * N, :],
                          in_=xdbl_full[b, XD + R + N:XD + R + 2 * N, mcs])
                      bcR = s_io.tile([2 * N, Tc], BF16, tag="bcR")
                      nc.vector.tensor_copy(out=bcR, in_=bcF[:, ::-1])
                      nc.sync.dma_start(out=brc_dram[:, par, :], in_=bcR)
                      for k in range(K):
                          if k == 1:
                              dtk, dtxc = dtRc, dtxRc
                              Bsrc = rep_dram(brc_dram, par * Tc,
                                              2 * Tc, N, DG, 1, Tc)
                              Csrc = rep_dram(brc_dram, (N * 2 + par) * Tc,
                                              2 * Tc, N, DG, 1, Tc)
                          elif True:
                              xf = xdbl_full[b]
                              Bsrc = rep_dram(xf, (k * XD + R) * L2 + ch * Tc,
                                              L2, N, DG, 1, Tc)
                              Csrc = rep_dram(xf, (k * XD + R + N) * L2 + ch * Tc,
                                              L2, N, DG, 1, Tc)
                              # dt chunk: matmul + sigmoid + ln
                              stg = s_io.tile([R, Tc], BF16, tag="sstg")
                              nc.sync.dma_start(
                                  out=stg, in_=xf[k * XD:k * XD + R, cs])
                              dps = s_d_ps.tile([Dsh, Tc], F32, tag="sdps")
                              mm(nc, dps, dtw_T[:, k, :], stg,
                                 start=True, stop=True)
                              sg = s_io.tile([Dsh, Tc], F32, tag="ssg")
                              nc.scalar.activation(out=sg, in_=dps,
                                                   func=AF.Copy if cfg.no_lut
                                                   else AF.Sigmoid,
                                                   bias=0.0 if cfg.no_lut
                                                   else dtb_s[:, k:k + 1],
                                                   scale=-1.0)
                              ki = 0 if k == 0 else 1
                              dtk = dt_st[:, par, ki, :]
                              nc.scalar.activation(out=dtk, in_=sg,
                                                   func=AF.Copy if cfg.no_lut
                                                   else AF.Ln)
                              dtxc = dtx_st[:, par, ki, :]
                              nc.vector.tensor_mul(dtxc, dtk, ibc)
                          if not cfg.no_rep:
                              B_rep = s_rep.tile([128, Tc], BF16, tag="brep")
                              nc.scalar.dma_start(out=B_rep, in_=Bsrc)
                              C_rep = s_rep.tile([128, Tc], BF16, tag="crep")
                              nc.scalar.dma_start(out=C_rep, in_=Csrc)
                          else:
                              B_rep = C_rep = dummy
                          for g in range(G):
                              gr = slice(g * DG, (g + 1) * DG)
                              ci = k * G + g
                              if cfg.no_rep:
                                  dtrep_ps, dtx_rep = (dummy,) * 2
                              else:
                                  st8 = s_io.tile([DG, Tc], BF16, tag="st8")
                                  nc.sync.dma_start(out=st8, in_=dtk[gr, :])
                                  dtrep_ps = s_dr_ps.tile([128, Tc], F32,
                                                          tag="dtrps")
                                  mm(nc, dtrep_ps, seldd_s, st8,
                                     start=True, stop=True)
                                  dtx_rep = s_rep.tile([128, Tc], BF16,
                                                       tag="dtxrep")
                                  nc.sync.dma_start(
                                      out=dtx_rep, in_=rep_sbuf(dtxc[gr, :], N))
                              a_t = s_a.tile([128, Tc], BF16, tag="a")
                              nc.scalar.activation(
                                  out=a_t, in_=dtrep_ps,
                                  func=AF.Copy if cfg.no_lut else AF.Exp,
                                  scale=A_col[:, ci:ci + 1])
                              b_t = s_b.tile([128, Tc], BF16, tag="b")
                              nc.vector.tensor_mul(b_t, dtx_rep, B_rep)
                              h_t = s_h.tile([128, Tc], BF16, tag="h")
                              init = 0.0 if ch == 0 else carry[:, ci:ci + 1]
                              if cfg.no_scan:
                                  nc.vector.tensor_mul(h_t, a_t, b_t)
                              else:
                                  nc.vector.tensor_tensor_scan(
                                      h_t, a_t, b_t, init, OP.mult, OP.add)
                              if ch < NCH - 1:
                                  nc.vector.tensor_copy(
                                      out=carry[:, ci:ci + 1],
                                      in_=h_t[:, Tc - 1:Tc])
                              hc = s_hc.tile([128, Tc], BF16, tag="hc")
                              nc.vector.tensor_mul(hc, h_t, C_rep)
                              if k == 1:
                                  mm(nc, y1_ps, red_s[:, g, :], hc,
                                     start=(g == 0), stop=(g == G - 1))
                              else:
                                  mm(nc, y02_ps, red_s[:, g, :], hc,
                                     start=(k == 0 and g == 0),
                                     stop=(k == K - 1 and g == G - 1))
                      # evac: yfin chunk = Dsum*inter + y02 ; y1 plain
                      yst = s_io.tile([Dsh, Tc], BF16, tag="yst")
                      nc.vector.scalar_tensor_tensor(
                          out=yst, in0=ibc, scalar=Dsum_s[:, 0:1],
                          in1=y02_ps, op0=OP.mult, op1=OP.add)
                      nc.sync.dma_start(out=yfin_dram[:, b, cs], in_=yst)
                      y1st = s_io.tile([Dsh, Tc], BF16, tag="y1st")
                      nc.scalar.copy(out=y1st, in_=y1_ps)
                      nc.sync.dma_start(out=y1_dram[:, b, cs], in_=y1st)

        # ============ Phase L: LN stats + allreduce + apply + out_proj ======
        # 4 (b,s) streams packed into partition quarters (rows bs*32+d)
        if "l" in cfg.phases:
          with tc.tile_pool(name="l_ps", bufs=2, space="PSUM") as l_ps, \
               tc.tile_pool(name="l_one", bufs=1) as l_one, \
               tc.tile_pool(name="l_sq", bufs=3) as l_sq, \
               tc.tile_pool(name="l_stg", bufs=4) as l_stg:
            ydp = l_one.tile([128, L], BF16, tag="ydp")
            MC = min(1024, L2)
            for b in range(B):
                for jj in range(L2 // MC):
                    yft = l_stg.tile([Dsh, MC], BF16, tag="yft")
                    nc.sync.dma_start(
                        out=yft, in_=yfin_dram[:, b, jj * MC:(jj + 1) * MC])
                    y1t = l_stg.tile([Dsh, MC], BF16, tag="y1t")
                    nc.sync.dma_start(
                        out=y1t,
                        in_=y1_dram[:, b, L2 - (jj + 1) * MC:L2 - jj * MC])
                    ym = l_stg.tile([Dsh, MC], BF16, tag="ym")
                    nc.vector.tensor_add(ym, yft, y1t[:, ::-1])
                    tok = slice(jj * MC // 2, (jj + 1) * MC // 2)
                    nc.vector.tensor_copy(
                        out=ydp[(2 * b) * Dsh:(2 * b + 1) * Dsh, tok],
                        in_=ym[:, 0::2])
                    nc.vector.tensor_copy(
                        out=ydp[(2 * b + 1) * Dsh:(2 * b + 2) * Dsh, tok],
                        in_=ym[:, 1::2])
            for j in range(nLch):
                js = slice(j * TcL, (j + 1) * TcL)
                sqp = l_sq.tile([128, TcL], BF16, tag="sqp")
                nc.vector.tensor_mul(sqp, ydp[:, js], ydp[:, js])
                sp = l_ps.tile([4 * B, TcL], F32, tag="sps")
                mm(nc, sp, selst[:, 0, :], ydp[:, js], start=True, stop=False)
                mm(nc, sp, selst[:, 1, :], sqp, start=False, stop=True)
                stg = l_stg.tile([4 * B, TcL], F32, tag="sstg2")
                nc.scalar.copy(out=stg, in_=sp)
                nc.sync.dma_start(
                    out=stats_part.rearrange("a x l -> (a x) l")[:, js], in_=stg)
            allreduce(stats_part, stats_full)
            # stats math, all on [64, SC] base-0 tiles
            s1f = l_one.tile([64, SC], F32, tag="s1f")
            s2f = l_one.tile([64, SC], F32, tag="s2f")
            flat = stats_full.rearrange("a x l -> (a x l)")
            half = 2 * B * L
            nc.sync.dma_start(
                out=s1f, in_=flat[0:half].rearrange("(p c) -> p c", p=64))
            nc.sync.dma_start(
                out=s2f, in_=flat[half:2 * half].rearrange("(p c) -> p c", p=64))
            mu_t = l_one.tile([64, SC], F32, tag="mu_t")
            var_t = l_one.tile([64, SC], F32, tag="var_t")
            musq = l_one.tile([64, SC], F32, tag="musq")
            eps_t = l_one.tile([64, 1], F32, tag="eps_t")
            nc.vector.memset(eps_t, 1e-5)
            nc.vector.tensor_scalar_mul(mu_t, s1f, 1.0 / D)
            nc.vector.tensor_scalar_mul(var_t, s2f, 1.0 / D)
            nc.vector.tensor_mul(musq, mu_t, mu_t)
            nc.vector.tensor_sub(var_t, var_t, musq)
            nc.scalar.activation(out=var_t, in_=var_t, func=AF.Sqrt, bias=eps_t)
            nc.vector.reciprocal(out=s1f, in_=var_t)          # alpha
            nc.vector.tensor_mul(s2f, mu_t, s1f)
            nc.vector.tensor_scalar_mul(s2f, s2f, -1.0)       # beta
            nc.sync.dma_start(
                out=ab_dram.rearrange("a x l -> (a x l)")[0:half].rearrange(
                    "(p c) -> p c", p=64), in_=s1f)
            nc.sync.dma_start(
                out=ab_dram.rearrange("a x l -> (a x l)")[half:2 * half].rearrange(
                    "(p c) -> p c", p=64), in_=s2f)

            # apply LN + gate + out_proj (packed ops, per-bs matmuls)
            with tc.tile_pool(name="l_rep", bufs=3) as l_rep, \
                 tc.tile_pool(name="l_t", bufs=3) as l_t, \
                 tc.tile_pool(name="o_st", bufs=2) as o_st, \
                 tc.tile_pool(name="o_ps", bufs=2, space="PSUM") as o_ps:
                for j in range(nLch):
                    js = slice(j * TcL, (j + 1) * TcL)
                    zcp = l_rep.tile([128, TcL], BF16, tag="zcp")
                    nc.sync.dma_start(
                        out=zcp,
                        in_=bass.AP(tensor=zs_dram.tensor,
                                    offset=zs_dram.offset + j * TcL,
                                    ap=[[L, 2 * B], [2 * B * L, Dsh],
                                        [1, TcL]]))
                    zwp = l_t.tile([128, TcL], BF16, tag="zwp")
                    bzp = l_t.tile([128, TcL], BF16, tag="bzp")
                    nc.vector.tensor_scalar_mul(zwp, zcp, lnwb4[:, 0:1])
                    nc.vector.tensor_scalar_mul(bzp, zcp, lnwb4[:, 1:2])
                    arp = l_rep.tile([128, TcL], F32, tag="arp")
                    brp = l_rep.tile([128, TcL], F32, tag="brp")
                    nc.sync.dma_start(
                        out=arp,
                        in_=bass.AP(tensor=ab_dram.tensor,
                                    offset=ab_dram.offset + j * TcL,
                                    ap=[[L, 2 * B], [0, Dsh], [1, TcL]]))
                    nc.scalar.dma_start(
                        out=brp,
                        in_=bass.AP(tensor=ab_dram.tensor,
                                    offset=ab_dram.offset + half + j * TcL,
                                    ap=[[L, 2 * B], [0, Dsh], [1, TcL]]))
                    t1 = l_t.tile([128, TcL], BF16, tag="t1")
                    nc.vector.tensor_mul(t1, ydp[:, js], arp)
                    t2 = l_t.tile([128, TcL], BF16, tag="t2")
                    nc.vector.tensor_add(t2, t1, brp)
                    t3 = l_t.tile([128, TcL], BF16, tag="t3")
                    nc.vector.tensor_mul(t3, t2, zwp)
                    fgp = l_t.tile([128, TcL], BF16, tag="fgp")
                    nc.vector.tensor_add(fgp, t3, bzp)
                    for bs in range(2 * B):
                        b, s = bs // 2, bs % 2
                        qs = slice(bs * Dsh, (bs + 1) * Dsh)
                        if bs * Dsh in (0, 32, 64):
                            lhs = opw4[qs, :]
                            rhs = fgp[qs, :]
                        else:
                            # base-96 quarter: stage to base 0 for the matmul
                            stq = o_st.tile([Dsh, TcL], BF16, tag="stq")
                            nc.sync.dma_start(out=stq, in_=fgp[qs, :])
                            lhs = opw_s[:, s, :]
                            rhs = stq
                        ops = o_ps.tile([C, TcL], F32, tag="ops")
                        mm(nc, ops, lhs, rhs, start=True, stop=True)
                        ost = o_st.tile([C, TcL], F32, tag="ost")
                        nc.scalar.copy(out=ost, in_=ops)
                        nc.sync.dma_start(out=outp_part[s, b, :, js], in_=ost)
              # ReduceScatter: rank r owns flat chunk r of [2,B,C,L]
            NCr = cfg.n_cores
            flat_n = 2 * B * C * L
            chunk = flat_n // NCr
            if cfg.fake_cc:
                nc.sync.dma_start(
                    out=outp_full.rearrange("s b c l -> (s b c l)"),
                    in_=outp_part.rearrange("s b c l -> (s b c l)"))
            else:
                nc.gpsimd.collective_compute(
                    "ReduceScatter", OP.add, replica_groups=groups,
                    ins=[outp_part.rearrange("s b c l -> (s b c l)").opt()],
                    outs=[outp_full.rearrange("s b c l -> (s b c l)")
                          [0:chunk].opt()])
            for b in range(B):
                nc.sync.dma_start(
                    out=out_ms[b].rearrange("c h w -> c (h w)"),
                    in_=outp_full[0, b])
                nc.sync.dma_start(
                    out=out_pan[b].rearrange("c h w -> c (h w)"),
                    in_=outp_full[1, b])

    nc.compile()
    return nc


# ---------------------------------------------------------------------------
# public entry point
# ---------------------------------------------------------------------------

_CACHE = {}


def _get_nc(cfg: Cfg):
    if cfg not in _CACHE:
        _CACHE[cfg] = build_nc(cfg)
    return _CACHE[cfg]


def kernel(**inputs):
    cfg = CFG
    nc = _get_nc(cfg)
    in_maps = host_prep(cfg, inputs)
    res = bass_utils.run_bass_kernel_spmd(
        nc, in_maps, core_ids=list(range(cfg.n_cores)))
    return assemble_outputs(cfg, res.results)


def assemble_outputs(cfg, results):
    """Each core's outputs hold its ReduceScatter chunk of the flat
    [2, B, C, L] result; stitch them back together."""
    B, C, L = cfg.B, cfg.C, cfg.L
    flat = np.zeros(2 * B * C * L, np.float32)
    chunk = flat.size // cfg.n_cores
    for r in range(cfg.n_cores):
        full_r = np.stack([
            np.asarray(results[r]["out_ms"], np.float32).reshape(B, C, L),
            np.asarray(results[r]["out_pan"], np.float32).reshape(B, C, L),
        ]).reshape(-1)
        # RS wrote rank r's reduced chunk to the FRONT of its local buffer
        flat[r * chunk:(r + 1) * chunk] = full_r[0:chunk]
    out = flat.reshape(2, B, C, cfg.H, cfg.W)
    return (out[0], out[1])

